# revision 4
# baseline (speedup 1.0000x reference)
"""Trainium2 Bass kernel for 3x3 Bayer demosaic (bilinear), batch-parallel
across 8 NeuronCores (one 1536x2048 image per core).

Algorithm (per image, RGGB-class Bayer patterns):
  feats: f0 = cross avg, f1 = diag avg, f2 = horiz avg, f3 = vert avg, f4 = x
  out[c, i, j] = feats[ind[c, i%2, j%2]][i, j]   (reflect-padded stencils)

Kernel layout: rows are split by parity into two SBUF tiles (E = even rows,
O = odd rows) so every vector op runs dense on the partition dim.  Vertical
neighbor sums become PE matmuls with a constant banded matrix (0.25 * (row m
+ row m+1)); horizontal sums are free-dim shifted adds on DVE.  Output
channels are assembled per (row-parity, col-parity) class with strided
column APs and DMA'd back with row-stride-2 DRAM access patterns.
"""

import sys

sys.path.insert(0, "/opt/trn_rl_repo")

import numpy as np

import concourse.bacc as bacc
import concourse.bass as bass
import concourse.tile as tile
from concourse import mybir
from concourse.bass_utils import run_bass_kernel_spmd

F32 = mybir.dt.float32

H, W = 1536, 2048
S = 192            # output rows per tile
M = S // 2         # 96  rows per parity half
K = M + 1          # 97  input rows loaded per parity half
NCH = 512          # psum column chunk
NCHUNKS = W // NCH

# Bayer phase tables (copied from the reference definition).
_IDX_RGGB = np.array([[4, 2], [3, 1], [0, 4], [4, 0], [1, 3], [2, 4]]).reshape(3, 2, 2)
IDX_MAPS = np.stack([
    _IDX_RGGB,
    np.roll(_IDX_RGGB, 1, axis=-1),
    np.roll(_IDX_RGGB, 1, axis=-2),
    np.roll(np.roll(_IDX_RGGB, 1, axis=-1), 1, axis=-2),
])  # (4, 3, 2, 2)
CODES = np.array([274, 4129, 4609, 8464], dtype=np.int32)


def _sh_quarter() -> np.ndarray:
    """lhsT [K, M]: out[m] = 0.25 * (rhs[m] + rhs[m+1])."""
    w = np.zeros((K, M), dtype=np.float32)
    for m in range(M):
        w[m, m] = 0.25
        w[m + 1, m] = 0.25
    return w


def _sh_quarter_halo() -> np.ndarray:
    """lhsT [K, M]: out[m] = 0.25 * (rhs[m-1] + rhs[m]), rhs[-1] := rhs[96].

    Used for the odd-row tile whose halo row (r0-1) is parked in
    partition 96 so all other consumers stay partition-0 aligned.
    """
    w = np.zeros((K, M), dtype=np.float32)
    for m in range(M):
        w[m, m] += 0.25
        w[m - 1 if m >= 1 else M, m] += 0.25
    return w


def build_program(ind: np.ndarray, height: int = H):
    """Build + compile the per-core Bass program for one Bayer phase map.

    ind: (3, 2, 2) int array, ind[c][row_parity][col_parity] in 0..4.
    """
    assert height % S == 0
    n_tiles = height // S

    nc = bacc.Bacc("TRN2", target_bir_lowering=False, debug=False)
    x = nc.dram_tensor("x", [height, W], F32, kind="ExternalInput")
    out = nc.dram_tensor("out", [3, height, W], F32, kind="ExternalOutput")
    shq = nc.dram_tensor("shq", [K, M], F32, kind="ExternalInput")
    shv = nc.dram_tensor("shv", [K, M], F32, kind="ExternalInput")
    xap = x.ap()
    oap = out.ap()

    with tile.TileContext(nc) as tc:
        with (
            tc.tile_pool(name="wpool", bufs=1) as wpool,
            tc.tile_pool(name="inp", bufs=2) as inp,
            tc.tile_pool(name="hsp", bufs=2) as hsp,
            tc.tile_pool(name="outp", bufs=2) as outp,
            tc.tile_pool(name="psum", bufs=2, space=bass.MemorySpace.PSUM) as psp,
        ):
            w_shq = wpool.tile([K, M], F32, tag="w_shq")
            nc.sync.dma_start(w_shq[:], shq.ap())
            w_shv = wpool.tile([K, M], F32, tag="w_shv")
            nc.sync.dma_start(w_shv[:], shv.ap())

            for t in range(n_tiles):
                r0 = S * t
                E = inp.tile([K, W], F32, tag="E")
                O = inp.tile([K, W], F32, tag="O")
                # E[m] = row r0 + 2m (m=0..96, reflect at bottom edge).
                # O[q] = row r0 + 1 + 2q (q=0..95); O[96] = halo row r0 - 1
                # (reflect -> row 1 for the first tile).
                if t == n_tiles - 1:
                    nc.sync.dma_start(E[0:M, :], xap[r0 : height : 2, :])
                    # reflect row `height` -> height - 2
                    nc.sync.dma_start(E[M:K, :], xap[height - 2 : height - 1, :])
                else:
                    nc.sync.dma_start(E[:], xap[r0 : r0 + 2 * K - 1 : 2, :])
                nc.sync.dma_start(O[0:M, :], xap[r0 + 1 : r0 + S : 2, :])
                halo = 1 if t == 0 else r0 - 1
                nc.sync.dma_start(O[M:K, :], xap[halo : halo + 1, :])

                # Horizontal neighbor sums, hs[:, j] = row[j-1] + row[j+1],
                # reflect at j=0 and j=W-1.
                hsE = hsp.tile([K, W], F32, tag="hsE")
                hsO = hsp.tile([K, W], F32, tag="hsO")
                for src, hs in ((E, hsE), (O, hsO)):
                    nc.vector.tensor_add(
                        hs[:, 1 : W - 1], src[:, 0 : W - 2], src[:, 2:W]
                    )
                    nc.vector.tensor_scalar_mul(hs[:, 0:1], src[:, 1:2], 2.0)
                    nc.vector.tensor_scalar_mul(
                        hs[:, W - 1 : W], src[:, W - 2 : W - 1], 2.0
                    )

                # Output tiles: [channel][row parity] -> [M, W]
                ot = [
                    [outp.tile([M, W], F32, tag=f"o{c}{p}", name=f"o{c}{p}") for p in range(2)]
                    for c in range(3)
                ]

                # x rows aligned with output rows, per parity side
                xrow = (E[0:M, :], O[0:M, :])
                hrow = (hsE[0:M, :], hsO[0:M, :])
                vsrc = (O, E)       # vertical-neighbor source per side
                dsrc = (hsO, hsE)   # diag = vertical sum of other side's hs
                vwgt = (w_shv, w_shq)  # halo-banded matrix for the O source

                # SBUF-sourced assembly (full width, strided cols).
                tog = [0]

                def eng_ts():
                    tog[0] ^= 1
                    return nc.vector if tog[0] else nc.scalar

                # GpSimd requires partition-start 0: E-side sources start at
                # partition 0, O-side at 1 -> route O-side to DVE/ACT.
                for c in range(3):
                    for p in range(2):
                        for q in range(2):
                            code = ind[c][p][q]
                            dst = ot[c][p][:, q::2]
                            if code == 4:
                                nc.gpsimd.tensor_copy(dst, xrow[p][:, q::2])
                            elif code == 2:
                                nc.scalar.mul(dst, hrow[p][:, q::2], 0.5)

                # PSUM chunked work: vertical / diagonal stencils via PE.
                for ci in range(NCHUNKS):
                    cs = slice(NCH * ci, NCH * (ci + 1))
                    v25 = []
                    ds = []
                    for p in range(2):
                        vt = psp.tile([M, NCH], F32, tag=f"v25{p}", name=f"v25{p}")
                        nc.tensor.matmul(
                            vt[:], vwgt[p][:], vsrc[p][:, cs], start=True, stop=True
                        )
                        v25.append(vt)
                    for p in range(2):
                        need_diag = any(ind[c][p][q] == 1 for c in range(3) for q in range(2))
                        if need_diag:
                            dt_ = psp.tile([M, NCH], F32, tag=f"ds{p}", name=f"ds{p}")
                            nc.tensor.matmul(
                                dt_[:], vwgt[p][:], dsrc[p][:, cs], start=True, stop=True
                            )
                            ds.append(dt_)
                        else:
                            ds.append(None)
                    for c in range(3):
                        for p in range(2):
                            for q in range(2):
                                code = ind[c][p][q]
                                dst = ot[c][p][:, NCH * ci + q : NCH * (ci + 1) : 2]
                                if code == 3:
                                    eng = eng_ts()
                                    if eng is nc.vector:
                                        nc.vector.tensor_scalar_mul(
                                            dst, v25[p][:, q::2], 2.0
                                        )
                                    else:
                                        nc.scalar.mul(dst, v25[p][:, q::2], 2.0)
                                elif code == 1:
                                    eng = eng_ts()
                                    if eng is nc.vector:
                                        nc.vector.tensor_copy(dst, ds[p][:, q::2])
                                    else:
                                        nc.scalar.copy(dst, ds[p][:, q::2])
                                elif code == 0:
                                    nc.vector.scalar_tensor_tensor(
                                        dst,
                                        hrow[p][:, NCH * ci + q : NCH * (ci + 1) : 2],
                                        0.25,
                                        v25[p][:, q::2],
                                        mybir.AluOpType.mult,
                                        mybir.AluOpType.add,
                                    )

                # Stores (row-parity de-interleave into DRAM).
                for c in range(3):
                    nc.sync.dma_start(oap[c, r0 : r0 + S : 2, :], ot[c][0][:])
                    nc.sync.dma_start(oap[c, r0 + 1 : r0 + S : 2, :], ot[c][1][:])

    nc.compile()
    return nc


_PROG_CACHE: dict = {}


def _get_program(sel: int, height: int = H):
    key = (sel, height)
    if key not in _PROG_CACHE:
        _PROG_CACHE[key] = build_program(IDX_MAPS[sel], height)
    return _PROG_CACHE[key]


def kernel(x: np.ndarray, bayer_mask: np.ndarray) -> np.ndarray:
    x = np.asarray(x)
    bayer_mask = np.asarray(bayer_mask)
    B, C, h, w = x.shape
    assert C == 1 and h == H and w == W, (B, C, h, w)

    p = bayer_mask[:, 0, :2, :2].astype(np.int64)
    code = p[:, 0, 0] * 4096 + p[:, 0, 1] * 256 + p[:, 1, 0] * 16 + p[:, 1, 1]
    sel = np.argmax(code[:, None] == CODES[None, :].astype(np.int64), axis=1)  # (B,)

    shq = _sh_quarter()
    shv = _sh_quarter_halo()
    result = np.empty((B, 3, h, w), dtype=np.float32)
    for s in np.unique(sel):
        idx = np.nonzero(sel == s)[0]
        nc = _get_program(int(s), h)
        in_maps = [
            {"x": np.ascontiguousarray(x[i, 0]), "shq": shq, "shv": shv}
            for i in idx
        ]
        res = run_bass_kernel_spmd(nc, in_maps, list(range(len(idx))))
        for j, i in enumerate(idx):
            result[i] = res.results[j]["out"]
    return result


# revision 5
# speedup vs baseline: 1.5402x; 1.5402x over previous
"""Trainium2 Bass kernel for 3x3 Bayer demosaic (bilinear), batch-parallel
across 8 NeuronCores (one 1536x2048 image per core).

Algorithm (per image, RGGB-class Bayer patterns):
  feats: f0 = cross avg, f1 = diag avg, f2 = horiz avg, f3 = vert avg, f4 = x
  out[c, i, j] = feats[ind[c, i%2, j%2]][i, j]   (reflect-padded stencils)

Layout: each SBUF partition holds a PAIR of image rows (even in columns
0..2047, odd in 2048..4095), so HBM transfers are fully contiguous with
16 KB per-partition descriptors and 96-descriptor (divisible-by-16)
transfers that spread across all 16 SDMA engines.  Vertical neighbor sums
are PE matmuls with constant banded [97, 96] matrices (the odd-row halo
row r0-1 is parked in partition 96, handled by a wrap-around band in the
matrix).  Horizontal sums are free-dim shifted adds on DVE.  The vertical
sums v25 = 0.25 * (up + down) are copied PSUM -> SBUF once and then feed
the vertical (f3 = 2 * v25), diagonal (f1 = v25[j-1] + v25[j+1]) and
cross (f0 = 0.25 * hs + v25) assemblies.  Output channels are assembled
per (row-parity, col-parity) class with strided column APs.
"""

import sys

sys.path.insert(0, "/opt/trn_rl_repo")

import numpy as np

import concourse.bacc as bacc
import concourse.bass as bass
import concourse.tile as tile
from concourse import mybir
from concourse.bass_utils import run_bass_kernel_spmd

F32 = mybir.dt.float32
AOP = mybir.AluOpType

H, W = 1536, 2048
S = 192            # output rows per tile
M = S // 2         # 96  row-pairs per tile
K = M + 1          # 97  partitions fed to the vertical matmuls
NCH = 1024         # psum column chunk (2 banks)
NCHUNKS = W // NCH

# Bayer phase tables (copied from the reference definition).
_IDX_RGGB = np.array([[4, 2], [3, 1], [0, 4], [4, 0], [1, 3], [2, 4]]).reshape(3, 2, 2)
IDX_MAPS = np.stack([
    _IDX_RGGB,
    np.roll(_IDX_RGGB, 1, axis=-1),
    np.roll(_IDX_RGGB, 1, axis=-2),
    np.roll(np.roll(_IDX_RGGB, 1, axis=-1), 1, axis=-2),
])  # (4, 3, 2, 2)
CODES = np.array([274, 4129, 4609, 8464], dtype=np.int32)


def _sh_quarter() -> np.ndarray:
    """lhsT [K, M]: out[m] = 0.25 * (rhs[m] + rhs[m+1]).  Even-row source."""
    w = np.zeros((K, M), dtype=np.float32)
    for m in range(M):
        w[m, m] = 0.25
        w[m + 1, m] = 0.25
    return w


def _sh_quarter_halo() -> np.ndarray:
    """lhsT [K, M]: out[m] = 0.25 * (rhs[m-1] + rhs[m]), rhs[-1] := rhs[96].

    Odd-row source whose halo row (r0-1) is parked in partition 96 so all
    other consumers stay partition-0 aligned.
    """
    w = np.zeros((K, M), dtype=np.float32)
    for m in range(M):
        w[m, m] += 0.25
        w[m - 1 if m >= 1 else M, m] += 0.25
    return w


def build_program(ind: np.ndarray, height: int = H):
    """Build + compile the per-core Bass program for one Bayer phase map.

    ind: (3, 2, 2) int array, ind[c][row_parity][col_parity] in 0..4.
    """
    assert height % S == 0
    n_tiles = height // S
    W2 = 2 * W  # 4096: free dim of the row-pair tiles

    nc = bacc.Bacc("TRN2", target_bir_lowering=False, debug=False)
    x = nc.dram_tensor("x", [height, W], F32, kind="ExternalInput")
    out = nc.dram_tensor("out", [3, height, W], F32, kind="ExternalOutput")
    shq = nc.dram_tensor("shq", [K, M], F32, kind="ExternalInput")
    shv = nc.dram_tensor("shv", [K, M], F32, kind="ExternalInput")
    xap = x.ap()
    oap = out.ap()

    with tile.TileContext(nc) as tc:
        with (
            tc.tile_pool(name="wpool", bufs=1) as wpool,
            tc.tile_pool(name="inp", bufs=2) as inp,
            tc.tile_pool(name="hsp", bufs=1) as hsp,
            tc.tile_pool(name="vcpp", bufs=2) as vcpp,
            tc.tile_pool(name="outp", bufs=2) as outp,
            tc.tile_pool(name="psum", bufs=2, space=bass.MemorySpace.PSUM) as psp,
        ):
            w_shq = wpool.tile([K, M], F32, tag="w_shq")
            nc.sync.dma_start(w_shq[:], shq.ap())
            w_shv = wpool.tile([K, M], F32, tag="w_shv")
            nc.sync.dma_start(w_shv[:], shv.ap())

            for t in range(n_tiles):
                r0 = S * t
                # X partition m: even half = row r0+2m, odd half = row r0+2m+1.
                # Partition 96: even half = row r0+192 (reflect at bottom),
                # odd half = halo row r0-1 (reflect at top).
                X = inp.tile([K, W2], F32, tag="X")
                nc.sync.dma_start(X[0:M, :], xap[r0 : r0 + S, :])
                ehalo = r0 + S if t < n_tiles - 1 else height - 2
                ohalo = r0 - 1 if t > 0 else 1
                nc.sync.dma_start(X[M:K, 0:W], xap[ehalo : ehalo + 1, :])
                nc.sync.dma_start(X[M:K, W:W2], xap[ohalo : ohalo + 1, :])

                # Horizontal neighbor sums for both halves in one op:
                # HS[:, j] = X[:, j-1] + X[:, j+1] per half, reflect at the
                # half edges.  Interior via a two-span 3D AP.
                HS = hsp.tile([K, W2], F32, tag="HS")
                x3 = X[:].rearrange("p (s c) -> p s c", s=2)
                h3 = HS[:].rearrange("p (s c) -> p s c", s=2)
                nc.vector.tensor_add(
                    h3[:, :, 1 : W - 1], x3[:, :, 0 : W - 2], x3[:, :, 2:W]
                )
                nc.vector.tensor_scalar_mul(h3[:, :, 0:1], x3[:, :, 1:2], 2.0)
                nc.vector.tensor_scalar_mul(
                    h3[:, :, W - 1 : W], x3[:, :, W - 2 : W - 1], 2.0
                )

                # Vertical sums v25 = 0.25*(up+down) via PE, then PSUM->SBUF.
                # vcp half 0 (cols 0..2047): for even output rows (src = odd
                # rows, wrap-around halo matrix); half 1: for odd output rows.
                vcp = vcpp.tile([M, W2], F32, tag="vcp")
                for ci in range(NCHUNKS):
                    lo = NCH * ci
                    for p in range(2):
                        vt = psp.tile([M, NCH], F32, tag=f"v{p}", name=f"v{p}")
                        src_half = W if p == 0 else 0  # vertical src: other parity
                        wgt = w_shv if p == 0 else w_shq
                        for h in range(NCH // 512):
                            c0 = src_half + lo + 512 * h
                            nc.tensor.matmul(
                                vt[:, 512 * h : 512 * (h + 1)],
                                wgt[:],
                                X[0:K, c0 : c0 + 512],
                                start=True,
                                stop=True,
                            )
                        nc.scalar.copy(vcp[:, p * W + lo : p * W + lo + NCH], vt[:])

                # Output tiles, one per channel, row-pair layout like X.
                ot = [outp.tile([M, W2], F32, tag=f"o{c}", name=f"o{c}") for c in range(3)]

                # Assembly: slot (c, p, q) covers output rows of parity p,
                # cols of parity q, from feature ind[c][p][q].
                nslot = 0
                for c in range(3):
                    for p in range(2):
                        hb = p * W  # half base for x/hs/out/vcp
                        for q in range(2):
                            code = ind[c][p][q]
                            dst = ot[c][:, hb + q : hb + W : 2]
                            nslot += 1
                            if code == 4:  # raw x
                                if nslot % 2:
                                    nc.gpsimd.tensor_copy(
                                        dst, X[0:M, hb + q : hb + W : 2]
                                    )
                                else:
                                    nc.scalar.copy(dst, X[0:M, hb + q : hb + W : 2])
                            elif code == 2:  # horizontal avg
                                nc.scalar.mul(dst, HS[0:M, hb + q : hb + W : 2], 0.5)
                            elif code == 3:  # vertical avg
                                nc.scalar.mul(dst, vcp[:, hb + q : hb + W : 2], 2.0)
                            elif code == 0:  # cross avg
                                nc.vector.scalar_tensor_tensor(
                                    dst,
                                    HS[0:M, hb + q : hb + W : 2],
                                    0.25,
                                    vcp[:, hb + q : hb + W : 2],
                                    AOP.mult,
                                    AOP.add,
                                )
                            elif code == 1:  # diagonal avg
                                if q == 0:
                                    nc.vector.tensor_add(
                                        ot[c][:, hb + 2 : hb + W : 2],
                                        vcp[:, hb + 1 : hb + W - 2 : 2],
                                        vcp[:, hb + 3 : hb + W : 2],
                                    )
                                    nc.vector.tensor_scalar_mul(
                                        ot[c][:, hb : hb + 1], vcp[:, hb + 1 : hb + 2], 2.0
                                    )
                                else:
                                    nc.vector.tensor_add(
                                        ot[c][:, hb + 1 : hb + W - 1 : 2],
                                        vcp[:, hb : hb + W - 3 : 2],
                                        vcp[:, hb + 2 : hb + W : 2],
                                    )
                                    nc.vector.tensor_scalar_mul(
                                        ot[c][:, hb + W - 1 : hb + W],
                                        vcp[:, hb + W - 2 : hb + W - 1],
                                        2.0,
                                    )

                for c in range(3):
                    nc.sync.dma_start(oap[c, r0 : r0 + S, :], ot[c][:])

    nc.compile()
    return nc


_PROG_CACHE: dict = {}


def _get_program(sel: int, height: int = H):
    key = (sel, height)
    if key not in _PROG_CACHE:
        _PROG_CACHE[key] = build_program(IDX_MAPS[sel], height)
    return _PROG_CACHE[key]


def kernel(x: np.ndarray, bayer_mask: np.ndarray) -> np.ndarray:
    x = np.asarray(x)
    bayer_mask = np.asarray(bayer_mask)
    B, C, h, w = x.shape
    assert C == 1 and h == H and w == W, (B, C, h, w)

    p = bayer_mask[:, 0, :2, :2].astype(np.int64)
    code = p[:, 0, 0] * 4096 + p[:, 0, 1] * 256 + p[:, 1, 0] * 16 + p[:, 1, 1]
    sel = np.argmax(code[:, None] == CODES[None, :].astype(np.int64), axis=1)  # (B,)

    shq = _sh_quarter()
    shv = _sh_quarter_halo()
    result = np.empty((B, 3, h, w), dtype=np.float32)
    for s in np.unique(sel):
        idx = np.nonzero(sel == s)[0]
        nc = _get_program(int(s), h)
        in_maps = [
            {"x": np.ascontiguousarray(x[i, 0]), "shq": shq, "shv": shv}
            for i in idx
        ]
        res = run_bass_kernel_spmd(nc, in_maps, list(range(len(idx))))
        for j, i in enumerate(idx):
            result[i] = res.results[j]["out"]
    return result


# revision 6
# speedup vs baseline: 1.9564x; 1.2702x over previous
"""Trainium2 Bass kernel for 3x3 Bayer demosaic (bilinear), batch-parallel
across 8 NeuronCores (one 1536x2048 image per core).

Algorithm (per image, RGGB-class Bayer patterns):
  feats: f0 = cross avg, f1 = diag avg, f2 = horiz avg, f3 = vert avg, f4 = x
  out[c, i, j] = feats[ind[c, i%2, j%2]][i, j]   (reflect-padded stencils)

Layout: each SBUF partition holds a PAIR of image rows (even in columns
0..2047, odd in 2048..4095), so HBM transfers are fully contiguous with
16 KB per-partition descriptors and 96-descriptor (divisible-by-16)
transfers that spread across all 16 SDMA engines.  Vertical neighbor sums
are PE matmuls with constant banded [97, 96] matrices (the odd-row halo
row r0-1 is parked in partition 96, handled by a wrap-around band in the
matrix).  Horizontal sums are free-dim shifted adds on DVE.  The vertical
sums v25 = 0.25 * (up + down) are copied PSUM -> SBUF once and then feed
the vertical (f3 = 2 * v25), diagonal (f1 = v25[j-1] + v25[j+1]) and
cross (f0 = 0.25 * hs + v25) assemblies.  Output channels are assembled
per (row-parity, col-parity) class with strided column APs.
"""

import sys

sys.path.insert(0, "/opt/trn_rl_repo")

import numpy as np

import concourse.bacc as bacc
import concourse.bass as bass
import concourse.tile as tile
from concourse import mybir
from concourse.bass_utils import run_bass_kernel_spmd

F32 = mybir.dt.float32
AOP = mybir.AluOpType

H, W = 1536, 2048
S = 192            # output rows per tile
M = S // 2         # 96  row-pairs per tile
K = M + 1          # 97  partitions fed to the vertical matmuls
NCH = 1024         # psum column chunk (2 banks)
NCHUNKS = W // NCH

# Bayer phase tables (copied from the reference definition).
_IDX_RGGB = np.array([[4, 2], [3, 1], [0, 4], [4, 0], [1, 3], [2, 4]]).reshape(3, 2, 2)
IDX_MAPS = np.stack([
    _IDX_RGGB,
    np.roll(_IDX_RGGB, 1, axis=-1),
    np.roll(_IDX_RGGB, 1, axis=-2),
    np.roll(np.roll(_IDX_RGGB, 1, axis=-1), 1, axis=-2),
])  # (4, 3, 2, 2)
CODES = np.array([274, 4129, 4609, 8464], dtype=np.int32)


def _sh_quarter() -> np.ndarray:
    """lhsT [K, M]: out[m] = 0.25 * (rhs[m] + rhs[m+1]).  Even-row source."""
    w = np.zeros((K, M), dtype=np.float32)
    for m in range(M):
        w[m, m] = 0.25
        w[m + 1, m] = 0.25
    return w


def _sh_quarter_halo() -> np.ndarray:
    """lhsT [K, M]: out[m] = 0.25 * (rhs[m-1] + rhs[m]), rhs[-1] := rhs[96].

    Odd-row source whose halo row (r0-1) is parked in partition 96 so all
    other consumers stay partition-0 aligned.
    """
    w = np.zeros((K, M), dtype=np.float32)
    for m in range(M):
        w[m, m] += 0.25
        w[m - 1 if m >= 1 else M, m] += 0.25
    return w


def build_program(ind: np.ndarray, height: int = H):
    """Build + compile the per-core Bass program for one Bayer phase map.

    ind: (3, 2, 2) int array, ind[c][row_parity][col_parity] in 0..4.
    """
    assert height % S == 0
    n_tiles = height // S
    W2 = 2 * W  # 4096: free dim of the row-pair tiles

    nc = bacc.Bacc("TRN2", target_bir_lowering=False, debug=False)
    x = nc.dram_tensor("x", [height, W], F32, kind="ExternalInput")
    out = nc.dram_tensor("out", [3, height, W], F32, kind="ExternalOutput")
    shq = nc.dram_tensor("shq", [K, M], F32, kind="ExternalInput")
    shv = nc.dram_tensor("shv", [K, M], F32, kind="ExternalInput")
    xap = x.ap()
    oap = out.ap()

    with tile.TileContext(nc) as tc:
        with (
            tc.tile_pool(name="wpool", bufs=1) as wpool,
            tc.tile_pool(name="inp", bufs=2) as inp,
            tc.tile_pool(name="hsp", bufs=1) as hsp,
            tc.tile_pool(name="vcpp", bufs=2) as vcpp,
            tc.tile_pool(name="outp", bufs=2) as outp,
            tc.tile_pool(name="psum", bufs=2, space=bass.MemorySpace.PSUM) as psp,
        ):
            w_shq = wpool.tile([K, M], F32, tag="w_shq")
            nc.scalar.dma_start(w_shq[:], shq.ap())
            w_shv = wpool.tile([K, M], F32, tag="w_shv")
            nc.scalar.dma_start(w_shv[:], shv.ap())

            for t in range(n_tiles):
                r0 = S * t
                # X partition m: even half = row r0+2m, odd half = row r0+2m+1.
                # Partition 96: even half = row r0+192 (reflect at bottom),
                # odd half = halo row r0-1 (reflect at top).
                X = inp.tile([K, W2], F32, tag="X")
                nc.scalar.dma_start(X[0:M, :], xap[r0 : r0 + S, :])
                ehalo = r0 + S if t < n_tiles - 1 else height - 2
                ohalo = r0 - 1 if t > 0 else 1
                nc.scalar.dma_start(X[M:K, 0:W], xap[ehalo : ehalo + 1, :])
                nc.scalar.dma_start(X[M:K, W:W2], xap[ohalo : ohalo + 1, :])

                # Horizontal neighbor sums for both halves in one op:
                # HS[:, j] = X[:, j-1] + X[:, j+1] per half, reflect at the
                # half edges.  Interior via a two-span 3D AP.
                HS = hsp.tile([K, W2], F32, tag="HS")
                for hb in (0, W):
                    nc.vector.tensor_add(
                        HS[:, hb + 1 : hb + W - 1],
                        X[:, hb : hb + W - 2],
                        X[:, hb + 2 : hb + W],
                    )
                    nc.scalar.mul(HS[:, hb : hb + 1], X[:, hb + 1 : hb + 2], 2.0)
                    nc.scalar.mul(
                        HS[:, hb + W - 1 : hb + W], X[:, hb + W - 2 : hb + W - 1], 2.0
                    )

                # Vertical sums v25 = 0.25*(up+down) via PE, then PSUM->SBUF.
                # vcp half 0 (cols 0..2047): for even output rows (src = odd
                # rows, wrap-around halo matrix); half 1: for odd output rows.
                vcp = vcpp.tile([M, W2], F32, tag="vcp")
                for ci in range(NCHUNKS):
                    lo = NCH * ci
                    for p in range(2):
                        vt = psp.tile([M, NCH], F32, tag=f"v{p}", name=f"v{p}")
                        src_half = W if p == 0 else 0  # vertical src: other parity
                        wgt = w_shv if p == 0 else w_shq
                        for h in range(NCH // 512):
                            c0 = src_half + lo + 512 * h
                            nc.tensor.matmul(
                                vt[:, 512 * h : 512 * (h + 1)],
                                wgt[:],
                                X[0:K, c0 : c0 + 512],
                                start=True,
                                stop=True,
                            )
                        nc.scalar.copy(vcp[:, p * W + lo : p * W + lo + NCH], vt[:])

                # Output tiles, one per channel, row-pair layout like X.
                ot = [outp.tile([M, W2], F32, tag=f"o{c}", name=f"o{c}") for c in range(3)]

                # Assembly: slot (c, p, q) covers output rows of parity p,
                # cols of parity q, from feature ind[c][p][q].
                nslot = 0
                for c in range(3):
                    for p in range(2):
                        hb = p * W  # half base for x/hs/out/vcp
                        for q in range(2):
                            code = ind[c][p][q]
                            dst = ot[c][:, hb + q : hb + W : 2]
                            nslot += 1
                            if code == 4:  # raw x
                                if nslot % 2:
                                    nc.gpsimd.tensor_copy(
                                        dst, X[0:M, hb + q : hb + W : 2]
                                    )
                                else:
                                    nc.scalar.copy(dst, X[0:M, hb + q : hb + W : 2])
                            elif code == 2:  # horizontal avg
                                nc.scalar.mul(dst, HS[0:M, hb + q : hb + W : 2], 0.5)
                            elif code == 3:  # vertical avg
                                nc.scalar.mul(dst, vcp[:, hb + q : hb + W : 2], 2.0)
                            elif code == 0:  # cross avg
                                nc.vector.scalar_tensor_tensor(
                                    dst,
                                    HS[0:M, hb + q : hb + W : 2],
                                    0.25,
                                    vcp[:, hb + q : hb + W : 2],
                                    AOP.mult,
                                    AOP.add,
                                )
                            elif code == 1:  # diagonal avg
                                if q == 0:
                                    nc.vector.tensor_add(
                                        ot[c][:, hb + 2 : hb + W : 2],
                                        vcp[:, hb + 1 : hb + W - 2 : 2],
                                        vcp[:, hb + 3 : hb + W : 2],
                                    )
                                    nc.vector.tensor_scalar_mul(
                                        ot[c][:, hb : hb + 1], vcp[:, hb + 1 : hb + 2], 2.0
                                    )
                                else:
                                    nc.vector.tensor_add(
                                        ot[c][:, hb + 1 : hb + W - 1 : 2],
                                        vcp[:, hb : hb + W - 3 : 2],
                                        vcp[:, hb + 2 : hb + W : 2],
                                    )
                                    nc.vector.tensor_scalar_mul(
                                        ot[c][:, hb + W - 1 : hb + W],
                                        vcp[:, hb + W - 2 : hb + W - 1],
                                        2.0,
                                    )

                for c in range(3):
                    nc.sync.dma_start(oap[c, r0 : r0 + S, :], ot[c][:])

    nc.compile()
    return nc


_PROG_CACHE: dict = {}


def _get_program(sel: int, height: int = H):
    key = (sel, height)
    if key not in _PROG_CACHE:
        _PROG_CACHE[key] = build_program(IDX_MAPS[sel], height)
    return _PROG_CACHE[key]


def kernel(x: np.ndarray, bayer_mask: np.ndarray) -> np.ndarray:
    x = np.asarray(x)
    bayer_mask = np.asarray(bayer_mask)
    B, C, h, w = x.shape
    assert C == 1 and h == H and w == W, (B, C, h, w)

    p = bayer_mask[:, 0, :2, :2].astype(np.int64)
    code = p[:, 0, 0] * 4096 + p[:, 0, 1] * 256 + p[:, 1, 0] * 16 + p[:, 1, 1]
    sel = np.argmax(code[:, None] == CODES[None, :].astype(np.int64), axis=1)  # (B,)

    shq = _sh_quarter()
    shv = _sh_quarter_halo()
    result = np.empty((B, 3, h, w), dtype=np.float32)
    for s in np.unique(sel):
        idx = np.nonzero(sel == s)[0]
        nc = _get_program(int(s), h)
        in_maps = [
            {"x": np.ascontiguousarray(x[i, 0]), "shq": shq, "shv": shv}
            for i in idx
        ]
        res = run_bass_kernel_spmd(nc, in_maps, list(range(len(idx))))
        for j, i in enumerate(idx):
            result[i] = res.results[j]["out"]
    return result


# revision 8
# speedup vs baseline: 2.1720x; 1.1102x over previous
"""Trainium2 Bass kernel for 3x3 Bayer demosaic (bilinear), batch-parallel
across 8 NeuronCores (one 1536x2048 image per core).

Algorithm (per image, RGGB-class Bayer patterns):
  feats: f0 = cross avg, f1 = diag avg, f2 = horiz avg, f3 = vert avg, f4 = x
  out[c, i, j] = feats[ind[c, i%2, j%2]][i, j]   (reflect-padded stencils)

Layout: each SBUF partition holds a PAIR of image rows (even in columns
0..2047, odd in 2048..4095), so HBM transfers are fully contiguous with
16 KB per-partition descriptors and 96-descriptor (divisible-by-16)
transfers that spread across all 16 SDMA engines.  Vertical neighbor sums
are PE matmuls with constant banded [97, 96] matrices (the odd-row halo
row r0-1 is parked in partition 96, handled by a wrap-around band in the
matrix).  Horizontal sums are free-dim shifted adds on DVE.  The vertical
sums v25 = 0.25 * (up + down) are copied PSUM -> SBUF once and then feed
the vertical (f3 = 2 * v25), diagonal (f1 = v25[j-1] + v25[j+1]) and
cross (f0 = 0.25 * hs + v25) assemblies.  Output channels are assembled
per (row-parity, col-parity) class with strided column APs.
"""

import sys

sys.path.insert(0, "/opt/trn_rl_repo")

import numpy as np

import concourse.bacc as bacc
import concourse.bass as bass
import concourse.tile as tile
from concourse import mybir
from concourse.bass_utils import run_bass_kernel_spmd

F32 = mybir.dt.float32
AOP = mybir.AluOpType

H, W = 1536, 2048
S = 192            # output rows per tile
M = S // 2         # 96  row-pairs per tile
K = M + 1          # 97  partitions fed to the vertical matmuls
NCH = 1024         # psum column chunk (2 banks)
NCHUNKS = W // NCH

# Bayer phase tables (copied from the reference definition).
_IDX_RGGB = np.array([[4, 2], [3, 1], [0, 4], [4, 0], [1, 3], [2, 4]]).reshape(3, 2, 2)
IDX_MAPS = np.stack([
    _IDX_RGGB,
    np.roll(_IDX_RGGB, 1, axis=-1),
    np.roll(_IDX_RGGB, 1, axis=-2),
    np.roll(np.roll(_IDX_RGGB, 1, axis=-1), 1, axis=-2),
])  # (4, 3, 2, 2)
CODES = np.array([274, 4129, 4609, 8464], dtype=np.int32)


def _sh_quarter() -> np.ndarray:
    """lhsT [K, M]: out[m] = 0.25 * (rhs[m] + rhs[m+1]).  Even-row source."""
    w = np.zeros((K, M), dtype=np.float32)
    for m in range(M):
        w[m, m] = 0.25
        w[m + 1, m] = 0.25
    return w


def _sh_quarter_halo() -> np.ndarray:
    """lhsT [K, M]: out[m] = 0.25 * (rhs[m-1] + rhs[m]), rhs[-1] := rhs[96].

    Odd-row source whose halo row (r0-1) is parked in partition 96 so all
    other consumers stay partition-0 aligned.
    """
    w = np.zeros((K, M), dtype=np.float32)
    for m in range(M):
        w[m, m] += 0.25
        w[m - 1 if m >= 1 else M, m] += 0.25
    return w


def build_program(ind: np.ndarray, height: int = H):
    """Build + compile the per-core Bass program for one Bayer phase map.

    ind: (3, 2, 2) int array, ind[c][row_parity][col_parity] in 0..4.
    """
    assert height % S == 0
    n_tiles = height // S
    W2 = 2 * W  # 4096: free dim of the row-pair tiles

    nc = bacc.Bacc("TRN2", target_bir_lowering=False, debug=False)
    x = nc.dram_tensor("x", [height, W], F32, kind="ExternalInput")
    out = nc.dram_tensor("out", [3, height, W], F32, kind="ExternalOutput")
    shq = nc.dram_tensor("shq", [K, M], F32, kind="ExternalInput")
    shv = nc.dram_tensor("shv", [K, M], F32, kind="ExternalInput")
    xap = x.ap()
    oap = out.ap()

    with tile.TileContext(nc) as tc:
        with (
            tc.tile_pool(name="wpool", bufs=1) as wpool,
            tc.tile_pool(name="inp", bufs=2) as inp,
            tc.tile_pool(name="hsp", bufs=1) as hsp,
            tc.tile_pool(name="vcpp", bufs=2) as vcpp,
            tc.tile_pool(name="outp", bufs=2) as outp,
            tc.tile_pool(name="psum", bufs=2, space=bass.MemorySpace.PSUM) as psp,
        ):
            w_shq = wpool.tile([K, M], F32, tag="w_shq")
            nc.scalar.dma_start(w_shq[:], shq.ap())
            w_shv = wpool.tile([K, M], F32, tag="w_shv")
            nc.scalar.dma_start(w_shv[:], shv.ap())

            for t in range(n_tiles):
                r0 = S * t
                # X partition m: even half = row r0+2m, odd half = row r0+2m+1.
                # Partition 96: even half = row r0+192 (reflect at bottom),
                # odd half = halo row r0-1 (reflect at top).
                X = inp.tile([K, W2], F32, tag="X")
                nc.scalar.dma_start(X[0:M, :], xap[r0 : r0 + S, :])
                ehalo = r0 + S if t < n_tiles - 1 else height - 2
                ohalo = r0 - 1 if t > 0 else 1
                nc.scalar.dma_start(X[M:K, 0:W], xap[ehalo : ehalo + 1, :])
                nc.scalar.dma_start(X[M:K, W:W2], xap[ohalo : ohalo + 1, :])

                # Horizontal neighbor sums for both halves in one op:
                # HS[:, j] = X[:, j-1] + X[:, j+1] per half, reflect at the
                # half edges.  Interior via a two-span 3D AP.
                HS = hsp.tile([K, W2], F32, tag="HS")
                for hb in (0, W):
                    nc.vector.tensor_add(
                        HS[:, hb + 1 : hb + W - 1],
                        X[:, hb : hb + W - 2],
                        X[:, hb + 2 : hb + W],
                    )
                    nc.scalar.mul(HS[:, hb : hb + 1], X[:, hb + 1 : hb + 2], 2.0)
                    nc.scalar.mul(
                        HS[:, hb + W - 1 : hb + W], X[:, hb + W - 2 : hb + W - 1], 2.0
                    )

                # Output tiles, one per channel, row-pair layout like X.
                ot = [outp.tile([M, W2], F32, tag=f"o{c}", name=f"o{c}") for c in range(3)]

                # Per side: which v25 column-parity the SBUF consumers
                # (f3 = code 3, f1 = code 1) need.  f0 reads PSUM directly.
                vpar = []
                for p in range(2):
                    need = set()
                    for c in range(3):
                        for q in range(2):
                            if ind[c][p][q] == 3:
                                need.add(q)
                            elif ind[c][p][q] == 1:
                                need.add(1 - q)
                    vpar.append(sorted(need))

                # Vertical sums v25 = 0.25*(up+down) via PE; copy only the
                # needed column parity PSUM->SBUF.  f0 slots consume the PSUM
                # chunks directly via scalar_tensor_tensor.
                vcp = vcpp.tile([M, W2], F32, tag="vcp")
                f0slots = [
                    (c, p, q)
                    for c in range(3)
                    for p in range(2)
                    for q in range(2)
                    if ind[c][p][q] == 0
                ]
                for ci in range(NCHUNKS):
                    lo = NCH * ci
                    for p in range(2):
                        vt = psp.tile([M, NCH], F32, tag=f"v{p}", name=f"v{p}")
                        src_half = W if p == 0 else 0  # vertical src: other parity
                        wgt = w_shv if p == 0 else w_shq
                        for h in range(NCH // 512):
                            c0 = src_half + lo + 512 * h
                            nc.tensor.matmul(
                                vt[:, 512 * h : 512 * (h + 1)],
                                wgt[:],
                                X[0:K, c0 : c0 + 512],
                                start=True,
                                stop=True,
                            )
                        base = p * W + lo
                        for q in vpar[p]:
                            nc.scalar.copy(
                                vcp[:, base + q : base + NCH : 2], vt[:, q::2]
                            )
                        for c, fp, fq in f0slots:
                            if fp != p:
                                continue
                            nc.vector.scalar_tensor_tensor(
                                ot[c][:, base + fq : base + NCH : 2],
                                HS[0:M, base + fq : base + NCH : 2],
                                0.25,
                                vt[:, fq::2],
                                AOP.mult,
                                AOP.add,
                            )

                # Assembly: slot (c, p, q) covers output rows of parity p,
                # cols of parity q, from feature ind[c][p][q].
                nslot = 0
                for c in range(3):
                    for p in range(2):
                        hb = p * W  # half base for x/hs/out/vcp
                        for q in range(2):
                            code = ind[c][p][q]
                            dst = ot[c][:, hb + q : hb + W : 2]
                            nslot += 1
                            if code == 4:  # raw x
                                if nslot % 2:
                                    nc.vector.tensor_copy(
                                        dst, X[0:M, hb + q : hb + W : 2]
                                    )
                                else:
                                    nc.scalar.copy(dst, X[0:M, hb + q : hb + W : 2])
                            elif code == 2:  # horizontal avg
                                nc.scalar.mul(dst, HS[0:M, hb + q : hb + W : 2], 0.5)
                            elif code == 3:  # vertical avg
                                nc.scalar.mul(dst, vcp[:, hb + q : hb + W : 2], 2.0)
                            elif code == 1:  # diagonal avg
                                if q == 0:
                                    nc.vector.tensor_add(
                                        ot[c][:, hb + 2 : hb + W : 2],
                                        vcp[:, hb + 1 : hb + W - 2 : 2],
                                        vcp[:, hb + 3 : hb + W : 2],
                                    )
                                    nc.vector.tensor_scalar_mul(
                                        ot[c][:, hb : hb + 1], vcp[:, hb + 1 : hb + 2], 2.0
                                    )
                                else:
                                    nc.vector.tensor_add(
                                        ot[c][:, hb + 1 : hb + W - 1 : 2],
                                        vcp[:, hb : hb + W - 3 : 2],
                                        vcp[:, hb + 2 : hb + W : 2],
                                    )
                                    nc.vector.tensor_scalar_mul(
                                        ot[c][:, hb + W - 1 : hb + W],
                                        vcp[:, hb + W - 2 : hb + W - 1],
                                        2.0,
                                    )

                for c in range(3):
                    nc.sync.dma_start(oap[c, r0 : r0 + S, :], ot[c][:])

    nc.compile()
    return nc


_PROG_CACHE: dict = {}


def _get_program(sel: int, height: int = H):
    key = (sel, height)
    if key not in _PROG_CACHE:
        _PROG_CACHE[key] = build_program(IDX_MAPS[sel], height)
    return _PROG_CACHE[key]


def kernel(x: np.ndarray, bayer_mask: np.ndarray) -> np.ndarray:
    x = np.asarray(x)
    bayer_mask = np.asarray(bayer_mask)
    B, C, h, w = x.shape
    assert C == 1 and h == H and w == W, (B, C, h, w)

    p = bayer_mask[:, 0, :2, :2].astype(np.int64)
    code = p[:, 0, 0] * 4096 + p[:, 0, 1] * 256 + p[:, 1, 0] * 16 + p[:, 1, 1]
    sel = np.argmax(code[:, None] == CODES[None, :].astype(np.int64), axis=1)  # (B,)

    shq = _sh_quarter()
    shv = _sh_quarter_halo()
    result = np.empty((B, 3, h, w), dtype=np.float32)
    for s in np.unique(sel):
        idx = np.nonzero(sel == s)[0]
        nc = _get_program(int(s), h)
        in_maps = [
            {"x": np.ascontiguousarray(x[i, 0]), "shq": shq, "shv": shv}
            for i in idx
        ]
        res = run_bass_kernel_spmd(nc, in_maps, list(range(len(idx))))
        for j, i in enumerate(idx):
            result[i] = res.results[j]["out"]
    return result
